# revision 27
# baseline (speedup 1.0000x reference)
"""AdaFocalLoss on 8 Trainium2 NeuronCores (Bass/Tile, SPMD).

Data-parallel over the batch axis: each core gets 8192 of the 65536
logit rows, the 15-entry gamma table is replicated, and the per-core
per-slot partial sums are combined on the host (the reduction over rows
is order-independent).

Per-core kernel structure (v6):
  - The 8192 rows form 64 (slot, partition) tiles of [128, 1000].  The
    stream is issued as multi-slot chunk DMAs with a partition-major
    host layout, so each partition line is one 4*k*1000-byte
    descriptor; 16KB descriptors measure ~400 GB/s HBM (4KB only ~345).
    Chunk sizes taper at both ends so compute starts early and little
    is exposed after the last byte.
  - ScalarE computes exp(x) (fp16 out; only engine with
    transcendentals).  Runs of slots whose row-sum goes to VectorE
    share ONE wide ACTIVATE (amortizes the fixed ~352-cycle cost); the
    remaining slots use per-slot ACTIVATEs whose accum_out produces the
    row-sum on ScalarE directly.  The split balances ScalarE (~74us)
    and VectorE (~73us) under the ~82us wire time.
  - The target logits are gathered by GpSimd indirect_copy (per 16-row
    partition group, index j of the group list = column idx[16q+j]):
    each chunk gathers [128, 16k] values of which the per-partition
    diagonal (i == p%16) is the true x_t; the tail extracts a whole
    part's diagonals with one broadcast multiply + reduce (~40ns/slot).
    No target sorting or window fallback is needed.
  - Tail per row: lse = ln(sumexp), logpt = x_t - lse, pt = exp(logpt);
    gamma's sign s and magnitude m come from one fused telescope pass
    (ge = pt >= thr over a packed [ds|dm] delta table, multiply,
    reduce); loss = -(1 + eps - s*pt)^m * logpt with the (1+eps, -1)
    affine folded into the Ln ACTIVATE.  Four parts (32/24/4/4 slots)
    so only the last part's serial chain is exposed past the stream.
  - Each part writes per-slot per-partition loss products into columns
    of a [128, 64] tile; one PE matmul against a ones vector reduces
    partitions and the host sums the [1, 64] outputs across cores (and
    negates).
"""

import sys

for _p in ("/opt/trn_rl_repo",):
    if _p not in sys.path:
        sys.path.insert(0, _p)

import numpy as np

NUM_BINS = 15
EPS = 1e-20
N, C = 65536, 1000
NCORES = 8
NSHARD = N // NCORES  # 8192 rows per core
P = 128  # SBUF partitions
R = NSHARD // P  # 64 row-slots per partition
G = 16  # indirect_copy gathers per-16-partition-group index lists

# chunk plan: tapered multi-slot DMAs (see module docstring)
CHUNKS = (
    [(0, 1), (1, 1), (2, 2), (4, 4)]
    + [(8 + 4 * i, 4) for i in range(12)]  # slots 8..55
    + [(56, 2), (58, 2), (60, 2), (62, 1), (63, 1)]
)
# slots whose row-sum comes from the ACTIVATE accum_out on ScalarE; the
# rest reduce on VectorE over the shared wide exp tile.  Mid-stream
# quads host most of them (the stream tail stays wide so ScalarE can
# race the wire there); the final slots take the short RD_ACC path.
A_SET = (
    {0, 1}
    | {s for q in range(8, 44, 4) for s in (q, q + 1)}
    | {62, 63}
)
# tail parts: only the small final part is exposed past the stream
TAIL_BOUNDS = [0, 32, 56, 64]
NPART = len(TAIL_BOUNDS) - 1
NMETA = 2 * NUM_BINS + NUM_BINS + G  # [ds|dm] + thr + diag mask

# per-chunk start column in the index table, 2-aligned (the Q7 reads the
# u16 index lists as 4-byte words)
IDX_POS = []
_p = 0
for _s0, _k in CHUNKS:
    IDX_POS.append(_p)
    _p += _k + (_k & 1)
NIDX = _p


def _split_excess_waits(nc, mybir, max_waits=1):
    """This container's walrus supports only one sync-wait command per
    instruction; hoist extra waits onto preceding same-engine no-ops."""
    ctr = 0
    for f in nc.m.functions:
        for bb in f.blocks:
            new_insts = []
            changed = False
            for inst in bb.instructions:
                si = inst.sync_info
                if si is not None and si.on_wait and len(si.on_wait) > max_waits:
                    waits = list(si.on_wait)
                    excess, keep = waits[:-max_waits], waits[-max_waits:]
                    for i in range(0, len(excess), max_waits):
                        ctr += 1
                        new_insts.append(
                            mybir.InstNoOp(
                                name=f"I-waitsplit-{ctr}",
                                sync_info=mybir.SyncInfo(
                                    on_wait=list(excess[i : i + max_waits]),
                                    on_update=[],
                                ),
                                bass_nofuse=True,
                                engine=inst.engine,
                            )
                        )
                    si.on_wait = keep
                    changed = True
                new_insts.append(inst)
            if changed:
                bb.instructions[:] = new_insts


def _build():
    import concourse.bass as bass
    import concourse.tile as tile
    from concourse import mybir

    f32 = mybir.dt.float32
    f16 = mybir.dt.float16
    u16 = mybir.dt.uint16
    AF = mybir.ActivationFunctionType
    ALU = mybir.AluOpType
    X = mybir.AxisListType.X
    NB = NUM_BINS

    nc = bass.Bass()
    x = nc.declare_dram_parameter("x", [NSHARD, C], f32, isOutput=False)
    meta = nc.declare_dram_parameter("meta", [P, NMETA], f32, isOutput=False)
    idx = nc.declare_dram_parameter("idx", [P, NIDX], u16, isOutput=False)
    out = nc.declare_dram_parameter("out", [1, R], f32, isOutput=True)

    x_ap = x[:]

    def slot_part(slot):
        h = 0
        while slot >= TAIL_BOUNDS[h + 1]:
            h += 1
        return h, slot - TAIL_BOUNDS[h]

    part_w = [TAIL_BOUNDS[h + 1] - TAIL_BOUNDS[h] for h in range(NPART)]

    with tile.TileContext(nc) as tc:
        with (
            tc.tile_pool(name="const", bufs=1) as cpool,
            tc.tile_pool(name="io", bufs=1) as iopool,
            tc.tile_pool(name="escr", bufs=1) as epool,
            tc.tile_pool(name="acc", bufs=1) as apool,
            tc.tile_pool(name="tail", bufs=2) as tpool,
            tc.tile_pool(name="psum", bufs=1, space="PSUM") as ppool,
        ):
            def chunk_dma(ci):
                s0, k = CHUNKS[ci]
                xt = iopool.tile(
                    [P, k * C], f32, tag=f"xt{k}",
                    name=f"xtile_c{ci}",
                    bufs=3 if k in (1, 4) else 2,
                )
                src = x_ap[s0 * P : (s0 + k) * P, :].rearrange(
                    "(p k) c -> p (k c)", p=P, k=k
                )
                nc.sync.dma_start(xt[:], src)
                return xt

            # the first two chunks stream before the constant loads so
            # compute can begin immediately
            early = {ci: chunk_dma(ci) for ci in range(2)}

            meta_t = cpool.tile([P, NMETA], f32, tag="meta")
            nc.sync.dma_start(meta_t[:], meta[:])
            dsm = meta_t[:, 0 : 2 * NB]
            thr = meta_t[:, 2 * NB : 3 * NB]
            mask16 = meta_t[:, 3 * NB : 3 * NB + G]
            idx_t = cpool.tile([P, NIDX], u16, tag="idxt")
            nc.sync.dma_start(idx_t[:], idx[:])
            ones = cpool.tile([P, 1], f32, tag="ones")

            # per-slot accumulators: row-sums of exp, gathered candidate
            # values (diagonal = target logit), per-slot loss products
            sumexp = apool.tile([P, R], f32, tag="sumexp")
            ps = ppool.tile([1, R], f32, tag="ps")
            xt_g = apool.tile([P, R * G], f32, tag="xt_g")
            prodcat = apool.tile([P, R], f32, tag="prodcat")

            def tail_part(h):
                F = part_w[h]
                off = TAIL_BOUNDS[h]
                # extract this part's target logits: per-partition
                # diagonal of the gathered [F, 16] groups
                gm = tpool.tile([P, F * G], f32, tag="gm", name=f"gm{h}")
                nc.vector.tensor_tensor(
                    gm[:].rearrange("p (f i) -> p f i", i=G),
                    xt_g[:, off * G : (off + F) * G].rearrange(
                        "p (f i) -> p f i", i=G
                    ),
                    mask16.rearrange("p (f i) -> p f i", f=1).broadcast_to(
                        [P, F, G]
                    ),
                    ALU.mult,
                )
                xt_p = tpool.tile([P, F], f32, tag="xt_p", name=f"xt_p{h}")
                nc.vector.tensor_reduce(
                    xt_p[:], gm[:].rearrange("p (f i) -> p f i", i=G), X, ALU.add
                )

                lse = tpool.tile([P, F], f32, tag="lse")
                nc.scalar.activation(lse[:], sumexp[:, off : off + F], AF.Ln)
                logpt = tpool.tile([P, F], f32, tag="logpt")
                nc.vector.tensor_sub(logpt[:], xt_p[:], lse[:])
                pt = tpool.tile([P, F], f32, tag="pt")
                nc.scalar.activation(pt[:], logpt[:], AF.Exp)

                # fused telescope: ge[p,f,j,b] = pt[p,f] >= thr[p,b],
                # prods = ge * [ds|dm][p,j,b], reduce b -> sm[p,f,j]
                ge = tpool.tile([P, F * 2 * NB], f32, tag="ge")
                ge4 = ge[:].rearrange("p (f j b) -> p f j b", j=2, b=NB)
                pt_b = (
                    pt[:]
                    .rearrange("p (f j b) -> p f j b", j=1, b=1)
                    .broadcast_to([P, F, 2, NB])
                )
                thr_b = thr.rearrange("p (f j b) -> p f j b", f=1, j=1).broadcast_to(
                    [P, F, 2, NB]
                )
                nc.vector.tensor_tensor(ge4, pt_b, thr_b, ALU.is_ge)
                dsm_b = dsm.rearrange("p (f j b) -> p f j b", f=1, j=2).broadcast_to(
                    [P, F, 2, NB]
                )
                prods = tpool.tile([P, F * 2 * NB], f32, tag="prods")
                nc.vector.tensor_tensor(
                    prods[:].rearrange("p (f j b) -> p f j b", j=2, b=NB),
                    ge4,
                    dsm_b,
                    ALU.mult,
                )
                sm = tpool.tile([P, F * 2], f32, tag="sm")
                nc.vector.tensor_reduce(
                    sm[:], prods[:].rearrange("p (f j b) -> p f j b", j=2, b=NB),
                    X, ALU.add,
                )
                sm3 = sm[:].rearrange("p (f j) -> p f j", j=2)
                pt3 = pt[:].rearrange("p (f j) -> p f j", j=1)

                # u = 1 + eps - s*pt ; y = u^m = exp(m * ln(u)); the
                # (scale=-1, bias=1+eps) affine rides the Ln ACTIVATE
                spt = tpool.tile([P, F], f32, tag="spt")
                nc.vector.tensor_mul(
                    spt[:].rearrange("p (f j) -> p f j", j=1), sm3[:, :, 0:1], pt3
                )
                v = tpool.tile([P, F], f32, tag="v")
                nc.scalar.activation(v[:], spt[:], AF.Ln, bias=1.0 + EPS, scale=-1.0)
                w_ = tpool.tile([P, F], f32, tag="w")
                nc.vector.tensor_mul(
                    w_[:].rearrange("p (f j) -> p f j", j=1),
                    v[:].rearrange("p (f j) -> p f j", j=1),
                    sm3[:, :, 1:2],
                )
                y = tpool.tile([P, F], f32, tag="y")
                nc.scalar.activation(y[:], w_[:], AF.Exp)
                # per-slot per-partition partial of sum y*logpt (negated
                # on host)
                nc.vector.tensor_mul(prodcat[:, off : off + F], y[:], logpt[:])

            done_parts = set()
            for ci, (s0, k) in enumerate(CHUNKS):
                xtile = early[ci] if ci in early else chunk_dma(ci)
                # gather the chunk's target-logit candidates on GpSimd
                ip = IDX_POS[ci]
                nc.gpsimd.indirect_copy(
                    xt_g[:, s0 * G : (s0 + k) * G],
                    xtile[:, 0 : k * C],
                    idx_t[:, ip : ip + k],
                    True,
                )
                # exps: runs of VectorE-summed slots share one wide
                # ACTIVATE; ScalarE-accum slots get their own
                j = 0
                while j < k:
                    s = s0 + j
                    if s in A_SET:
                        eo = epool.tile([P, C], f16, tag="eo", bufs=3)
                        nc.scalar.activation(
                            eo[:],
                            xtile[:, j * C : (j + 1) * C],
                            AF.Exp,
                            accum_out=sumexp[:, s : s + 1],
                        )
                        j += 1
                    else:
                        j2 = j
                        while j2 < k and (s0 + j2) not in A_SET:
                            j2 += 1
                        wdt = j2 - j
                        eow = epool.tile(
                            [P, wdt * C], f16, tag=f"eow{wdt}", bufs=3, name="eow"
                        )
                        nc.scalar.activation(
                            eow[:], xtile[:, j * C : j2 * C], AF.Exp
                        )
                        for jj in range(j, j2):
                            nc.vector.tensor_reduce(
                                sumexp[:, s0 + jj : s0 + jj + 1],
                                eow[:, (jj - j) * C : (jj - j + 1) * C],
                                X,
                                ALU.add,
                            )
                        j = j2
                if ci == 1:
                    nc.vector.memset(ones[:], 1.0)
                hi = s0 + k - 1
                for h in range(NPART):
                    if hi >= TAIL_BOUNDS[h + 1] - 1 and h not in done_parts:
                        done_parts.add(h)
                        tail_part(h)  # all but the last overlap the stream
                        if h == NPART - 2:
                            # flush the finished slots' partition-reduce
                            # and output DMA under the stream
                            E = TAIL_BOUNDS[NPART - 1]
                            nc.tensor.matmul(
                                ps[:, 0:E], ones[:], prodcat[:, 0:E],
                                start=True, stop=True,
                            )
                            res1 = tpool.tile([1, E], f32, tag="res1")
                            nc.scalar.copy(res1[:], ps[:, 0:E])
                            nc.sync.dma_start(out[:, 0:E], res1[:])

            E = TAIL_BOUNDS[NPART - 1]
            nc.tensor.matmul(
                ps[:, E:R], ones[:], prodcat[:, E:R], start=True, stop=True
            )
            res2 = tpool.tile([1, R - E], f32, tag="res2")
            nc.scalar.copy(res2[:], ps[:, E:R])
            nc.sync.dma_start(out[:, E:R], res2[:])

    _split_excess_waits(nc, mybir, max_waits=1)
    return nc


_NC_CACHE = {}


def _get_nc():
    if "nc" not in _NC_CACHE:
        _NC_CACHE["nc"] = _build()
    return _NC_CACHE["nc"]


def _make_in_maps(input, target, gammas):
    inp = np.ascontiguousarray(np.asarray(input, dtype=np.float32))
    tgt = np.asarray(target).astype(np.int64)
    gam = np.asarray(gammas, dtype=np.float32)
    assert inp.shape == (N, C) and tgt.shape == (N,) and gam.shape == (NUM_BINS,)

    # packed constants: telescoped [ds|dm] gamma deltas, bin thresholds,
    # and the per-partition diagonal mask for the grouped gather
    sgn, mag = np.sign(gam), np.abs(gam)
    ds = np.concatenate([sgn[:1], sgn[1:] - sgn[:-1]])
    dm = np.concatenate([mag[:1], mag[1:] - mag[:-1]])
    thr = np.arange(NUM_BINS, dtype=np.float32) / NUM_BINS
    mrow = np.concatenate([ds, dm, thr]).astype(np.float32)
    meta = np.zeros((P, NMETA), dtype=np.float32)
    meta[:, : 3 * NUM_BINS] = mrow
    meta[np.arange(P), 3 * NUM_BINS + (np.arange(P) % G)] = 1.0

    in_maps = []
    for i in range(NCORES):
        shard = inp[NSHARD * i : NSHARD * (i + 1)]
        tsh = tgt[NSHARD * i : NSHARD * (i + 1)].reshape(R, P)  # [slot, partition]
        # chunk layout: within a k-slot chunk, partition-major so each
        # partition line is one contiguous 4*k*C-byte DMA descriptor
        xbuf = shard.copy()
        for s0, k in CHUNKS:
            if k > 1:
                xbuf[s0 * P : (s0 + k) * P] = (
                    shard[s0 * P : (s0 + k) * P]
                    .reshape(k, P, C)
                    .transpose(1, 0, 2)
                    .reshape(k * P, C)
                )
        # gather indices: slot s (at position j of its chunk) gathers
        # column j*C + target
        off = np.empty(R, dtype=np.int64)
        for s0, k in CHUNKS:
            for j in range(k):
                off[s0 + j] = j * C
        idxv = np.zeros((P, NIDX), dtype=np.uint16)
        for ci, (s0, k) in enumerate(CHUNKS):
            ip = IDX_POS[ci]
            idxv[:, ip : ip + k] = tsh[s0 : s0 + k].T + off[None, s0 : s0 + k]
        in_maps.append(
            {
                "x": np.ascontiguousarray(xbuf),
                "meta": meta,
                "idx": np.ascontiguousarray(idxv),
            }
        )
    return in_maps


def kernel(input, target, gammas, _trace=False, _tmpdir=None):
    from concourse.bass_utils import run_bass_kernel_spmd

    in_maps = _make_in_maps(input, target, gammas)
    res = run_bass_kernel_spmd(
        _get_nc(),
        in_maps,
        core_ids=list(range(NCORES)),
        trace=_trace,
        tmpdir=_tmpdir,
    )
    total = 0.0
    for i in range(NCORES):
        total += float(np.asarray(res.results[i]["out"], dtype=np.float64).sum())
    if _trace:
        kernel._last_result = res
    return np.array(-total, dtype=np.float32)


# revision 28
# speedup vs baseline: 1.0197x; 1.0197x over previous
"""AdaFocalLoss on 8 Trainium2 NeuronCores (Bass/Tile, SPMD).

Data-parallel over the batch axis: each core gets 8192 of the 65536
logit rows, the 15-entry gamma table is replicated, and the per-core
per-slot partial sums are combined on the host (the reduction over rows
is order-independent).

Per-core kernel structure (v6):
  - The 8192 rows form 64 (slot, partition) tiles of [128, 1000].  The
    stream is issued as multi-slot chunk DMAs with a partition-major
    host layout, so each partition line is one 4*k*1000-byte
    descriptor; 16KB descriptors measure ~400 GB/s HBM (4KB only ~345).
    Chunk sizes taper at both ends so compute starts early and little
    is exposed after the last byte.
  - ScalarE computes exp(x) (fp16 out; only engine with
    transcendentals).  Runs of slots whose row-sum goes to VectorE
    share ONE wide ACTIVATE (amortizes the fixed ~352-cycle cost); the
    remaining slots use per-slot ACTIVATEs whose accum_out produces the
    row-sum on ScalarE directly.  The split balances ScalarE (~74us)
    and VectorE (~73us) under the ~82us wire time.
  - The target logits are gathered by GpSimd indirect_copy (per 16-row
    partition group, index j of the group list = column idx[16q+j]):
    each chunk gathers [128, 16k] values of which the per-partition
    diagonal (i == p%16) is the true x_t; the tail extracts a whole
    part's diagonals with one broadcast multiply + reduce (~40ns/slot).
    No target sorting or window fallback is needed.
  - Tail per row: lse = ln(sumexp), logpt = x_t - lse, pt = exp(logpt);
    gamma's sign s and magnitude m come from one fused telescope pass
    (ge = pt >= thr over a packed [ds|dm] delta table, multiply,
    reduce); loss = -(1 + eps - s*pt)^m * logpt with the (1+eps, -1)
    affine folded into the Ln ACTIVATE.  Four parts (32/24/4/4 slots)
    so only the last part's serial chain is exposed past the stream.
  - Each part writes per-slot per-partition loss products into columns
    of a [128, 64] tile; one PE matmul against a ones vector reduces
    partitions and the host sums the [1, 64] outputs across cores (and
    negates).
"""

import sys

for _p in ("/opt/trn_rl_repo",):
    if _p not in sys.path:
        sys.path.insert(0, _p)

import numpy as np

NUM_BINS = 15
EPS = 1e-20
N, C = 65536, 1000
NCORES = 8
NSHARD = N // NCORES  # 8192 rows per core
P = 128  # SBUF partitions
R = NSHARD // P  # 64 row-slots per partition
G = 16  # indirect_copy gathers per-16-partition-group index lists

# chunk plan: tapered multi-slot DMAs (see module docstring)
CHUNKS = (
    [(0, 1), (1, 1), (2, 2), (4, 4)]
    + [(8 + 4 * i, 4) for i in range(12)]  # slots 8..55
    + [(56, 2), (58, 2), (60, 2), (62, 1), (63, 1)]
)
# slots whose row-sum comes from the ACTIVATE accum_out on ScalarE; the
# rest reduce on VectorE over the shared wide exp tile.  Mid-stream
# quads host most of them (the stream tail stays wide so ScalarE can
# race the wire there); the final slots take the short RD_ACC path.
A_SET = (
    {0, 1}
    | {s for q in range(8, 40, 4) for s in (q, q + 1)}
    | {60, 61, 62, 63}
)
# tail parts: only the small final part is exposed past the stream
TAIL_BOUNDS = [0, 32, 56, 64]
NPART = len(TAIL_BOUNDS) - 1
NMETA = 2 * NUM_BINS + NUM_BINS + G  # [ds|dm] + thr + diag mask

# per-chunk start column in the index table, 2-aligned (the Q7 reads the
# u16 index lists as 4-byte words)
IDX_POS = []
_p = 0
for _s0, _k in CHUNKS:
    IDX_POS.append(_p)
    _p += _k + (_k & 1)
NIDX = _p


def _split_excess_waits(nc, mybir, max_waits=1):
    """This container's walrus supports only one sync-wait command per
    instruction; hoist extra waits onto preceding same-engine no-ops."""
    ctr = 0
    for f in nc.m.functions:
        for bb in f.blocks:
            new_insts = []
            changed = False
            for inst in bb.instructions:
                si = inst.sync_info
                if si is not None and si.on_wait and len(si.on_wait) > max_waits:
                    waits = list(si.on_wait)
                    excess, keep = waits[:-max_waits], waits[-max_waits:]
                    for i in range(0, len(excess), max_waits):
                        ctr += 1
                        new_insts.append(
                            mybir.InstNoOp(
                                name=f"I-waitsplit-{ctr}",
                                sync_info=mybir.SyncInfo(
                                    on_wait=list(excess[i : i + max_waits]),
                                    on_update=[],
                                ),
                                bass_nofuse=True,
                                engine=inst.engine,
                            )
                        )
                    si.on_wait = keep
                    changed = True
                new_insts.append(inst)
            if changed:
                bb.instructions[:] = new_insts


def _build():
    import concourse.bass as bass
    import concourse.tile as tile
    from concourse import mybir

    f32 = mybir.dt.float32
    f16 = mybir.dt.float16
    u16 = mybir.dt.uint16
    AF = mybir.ActivationFunctionType
    ALU = mybir.AluOpType
    X = mybir.AxisListType.X
    NB = NUM_BINS

    nc = bass.Bass()
    x = nc.declare_dram_parameter("x", [NSHARD, C], f32, isOutput=False)
    meta = nc.declare_dram_parameter("meta", [P, NMETA], f32, isOutput=False)
    idx = nc.declare_dram_parameter("idx", [P, NIDX], u16, isOutput=False)
    out = nc.declare_dram_parameter("out", [1, R], f32, isOutput=True)

    x_ap = x[:]

    def slot_part(slot):
        h = 0
        while slot >= TAIL_BOUNDS[h + 1]:
            h += 1
        return h, slot - TAIL_BOUNDS[h]

    part_w = [TAIL_BOUNDS[h + 1] - TAIL_BOUNDS[h] for h in range(NPART)]

    with tile.TileContext(nc) as tc:
        with (
            tc.tile_pool(name="const", bufs=1) as cpool,
            tc.tile_pool(name="io", bufs=1) as iopool,
            tc.tile_pool(name="escr", bufs=1) as epool,
            tc.tile_pool(name="acc", bufs=1) as apool,
            tc.tile_pool(name="tail", bufs=2) as tpool,
            tc.tile_pool(name="psum", bufs=1, space="PSUM") as ppool,
        ):
            def chunk_dma(ci):
                s0, k = CHUNKS[ci]
                xt = iopool.tile(
                    [P, k * C], f32, tag=f"xt{k}",
                    name=f"xtile_c{ci}",
                    bufs=3 if k in (1, 4) else 2,
                )
                src = x_ap[s0 * P : (s0 + k) * P, :].rearrange(
                    "(p k) c -> p (k c)", p=P, k=k
                )
                nc.sync.dma_start(xt[:], src)
                return xt

            # the first two chunks stream before the constant loads so
            # compute can begin immediately
            early = {ci: chunk_dma(ci) for ci in range(2)}

            meta_t = cpool.tile([P, NMETA], f32, tag="meta")
            nc.sync.dma_start(meta_t[:], meta[:])
            dsm = meta_t[:, 0 : 2 * NB]
            thr = meta_t[:, 2 * NB : 3 * NB]
            mask16 = meta_t[:, 3 * NB : 3 * NB + G]
            idx_t = cpool.tile([P, NIDX], u16, tag="idxt")
            nc.sync.dma_start(idx_t[:], idx[:])
            ones = cpool.tile([P, 1], f32, tag="ones")

            # per-slot accumulators: row-sums of exp, gathered candidate
            # values (diagonal = target logit), per-slot loss products
            sumexp = apool.tile([P, R], f32, tag="sumexp")
            ps = ppool.tile([1, R], f32, tag="ps")
            xt_g = apool.tile([P, R * G], f32, tag="xt_g")
            prodcat = apool.tile([P, R], f32, tag="prodcat")

            def tail_part(h):
                F = part_w[h]
                off = TAIL_BOUNDS[h]
                # extract this part's target logits: per-partition
                # diagonal of the gathered [F, 16] groups
                gm = tpool.tile([P, F * G], f32, tag="gm", name=f"gm{h}")
                nc.vector.tensor_tensor(
                    gm[:].rearrange("p (f i) -> p f i", i=G),
                    xt_g[:, off * G : (off + F) * G].rearrange(
                        "p (f i) -> p f i", i=G
                    ),
                    mask16.rearrange("p (f i) -> p f i", f=1).broadcast_to(
                        [P, F, G]
                    ),
                    ALU.mult,
                )
                xt_p = tpool.tile([P, F], f32, tag="xt_p", name=f"xt_p{h}")
                nc.vector.tensor_reduce(
                    xt_p[:], gm[:].rearrange("p (f i) -> p f i", i=G), X, ALU.add
                )

                lse = tpool.tile([P, F], f32, tag="lse")
                nc.scalar.activation(lse[:], sumexp[:, off : off + F], AF.Ln)
                logpt = tpool.tile([P, F], f32, tag="logpt")
                nc.vector.tensor_sub(logpt[:], xt_p[:], lse[:])
                pt = tpool.tile([P, F], f32, tag="pt")
                nc.scalar.activation(pt[:], logpt[:], AF.Exp)

                # fused telescope: ge[p,f,j,b] = pt[p,f] >= thr[p,b],
                # prods = ge * [ds|dm][p,j,b], reduce b -> sm[p,f,j]
                ge = tpool.tile([P, F * 2 * NB], f32, tag="ge")
                ge4 = ge[:].rearrange("p (f j b) -> p f j b", j=2, b=NB)
                pt_b = (
                    pt[:]
                    .rearrange("p (f j b) -> p f j b", j=1, b=1)
                    .broadcast_to([P, F, 2, NB])
                )
                thr_b = thr.rearrange("p (f j b) -> p f j b", f=1, j=1).broadcast_to(
                    [P, F, 2, NB]
                )
                nc.vector.tensor_tensor(ge4, pt_b, thr_b, ALU.is_ge)
                dsm_b = dsm.rearrange("p (f j b) -> p f j b", f=1, j=2).broadcast_to(
                    [P, F, 2, NB]
                )
                prods = tpool.tile([P, F * 2 * NB], f32, tag="prods")
                nc.vector.tensor_tensor(
                    prods[:].rearrange("p (f j b) -> p f j b", j=2, b=NB),
                    ge4,
                    dsm_b,
                    ALU.mult,
                )
                sm = tpool.tile([P, F * 2], f32, tag="sm")
                nc.vector.tensor_reduce(
                    sm[:], prods[:].rearrange("p (f j b) -> p f j b", j=2, b=NB),
                    X, ALU.add,
                )
                sm3 = sm[:].rearrange("p (f j) -> p f j", j=2)
                pt3 = pt[:].rearrange("p (f j) -> p f j", j=1)

                # u = 1 + eps - s*pt ; y = u^m = exp(m * ln(u)); the
                # (scale=-1, bias=1+eps) affine rides the Ln ACTIVATE
                spt = tpool.tile([P, F], f32, tag="spt")
                nc.vector.tensor_mul(
                    spt[:].rearrange("p (f j) -> p f j", j=1), sm3[:, :, 0:1], pt3
                )
                v = tpool.tile([P, F], f32, tag="v")
                nc.scalar.activation(v[:], spt[:], AF.Ln, bias=1.0 + EPS, scale=-1.0)
                w_ = tpool.tile([P, F], f32, tag="w")
                nc.vector.tensor_mul(
                    w_[:].rearrange("p (f j) -> p f j", j=1),
                    v[:].rearrange("p (f j) -> p f j", j=1),
                    sm3[:, :, 1:2],
                )
                y = tpool.tile([P, F], f32, tag="y")
                nc.scalar.activation(y[:], w_[:], AF.Exp)
                # per-slot per-partition partial of sum y*logpt (negated
                # on host)
                nc.vector.tensor_mul(prodcat[:, off : off + F], y[:], logpt[:])

            done_parts = set()
            for ci, (s0, k) in enumerate(CHUNKS):
                xtile = early[ci] if ci in early else chunk_dma(ci)
                # gather the chunk's target-logit candidates on GpSimd
                ip = IDX_POS[ci]
                nc.gpsimd.indirect_copy(
                    xt_g[:, s0 * G : (s0 + k) * G],
                    xtile[:, 0 : k * C],
                    idx_t[:, ip : ip + k],
                    True,
                )
                # exps: runs of VectorE-summed slots share one wide
                # ACTIVATE; ScalarE-accum slots get their own
                j = 0
                while j < k:
                    s = s0 + j
                    if s in A_SET:
                        eo = epool.tile([P, C], f16, tag="eo", bufs=3)
                        nc.scalar.activation(
                            eo[:],
                            xtile[:, j * C : (j + 1) * C],
                            AF.Exp,
                            accum_out=sumexp[:, s : s + 1],
                        )
                        j += 1
                    else:
                        j2 = j
                        while j2 < k and (s0 + j2) not in A_SET:
                            j2 += 1
                        wdt = j2 - j
                        eow = epool.tile(
                            [P, wdt * C], f16, tag=f"eow{wdt}", bufs=3, name="eow"
                        )
                        nc.scalar.activation(
                            eow[:], xtile[:, j * C : j2 * C], AF.Exp
                        )
                        for jj in range(j, j2):
                            nc.vector.tensor_reduce(
                                sumexp[:, s0 + jj : s0 + jj + 1],
                                eow[:, (jj - j) * C : (jj - j + 1) * C],
                                X,
                                ALU.add,
                            )
                        j = j2
                if ci == 1:
                    nc.vector.memset(ones[:], 1.0)
                hi = s0 + k - 1
                for h in range(NPART):
                    if hi >= TAIL_BOUNDS[h + 1] - 1 and h not in done_parts:
                        done_parts.add(h)
                        tail_part(h)  # all but the last overlap the stream
                        if h == NPART - 2:
                            # flush the finished slots' partition-reduce
                            # and output DMA under the stream
                            E = TAIL_BOUNDS[NPART - 1]
                            nc.tensor.matmul(
                                ps[:, 0:E], ones[:], prodcat[:, 0:E],
                                start=True, stop=True,
                            )
                            res1 = tpool.tile([1, E], f32, tag="res1")
                            nc.scalar.copy(res1[:], ps[:, 0:E])
                            nc.sync.dma_start(out[:, 0:E], res1[:])

            E = TAIL_BOUNDS[NPART - 1]
            nc.tensor.matmul(
                ps[:, E:R], ones[:], prodcat[:, E:R], start=True, stop=True
            )
            res2 = tpool.tile([1, R - E], f32, tag="res2")
            nc.scalar.copy(res2[:], ps[:, E:R])
            nc.sync.dma_start(out[:, E:R], res2[:])

    _split_excess_waits(nc, mybir, max_waits=1)
    return nc


_NC_CACHE = {}


def _get_nc():
    if "nc" not in _NC_CACHE:
        _NC_CACHE["nc"] = _build()
    return _NC_CACHE["nc"]


def _make_in_maps(input, target, gammas):
    inp = np.ascontiguousarray(np.asarray(input, dtype=np.float32))
    tgt = np.asarray(target).astype(np.int64)
    gam = np.asarray(gammas, dtype=np.float32)
    assert inp.shape == (N, C) and tgt.shape == (N,) and gam.shape == (NUM_BINS,)

    # packed constants: telescoped [ds|dm] gamma deltas, bin thresholds,
    # and the per-partition diagonal mask for the grouped gather
    sgn, mag = np.sign(gam), np.abs(gam)
    ds = np.concatenate([sgn[:1], sgn[1:] - sgn[:-1]])
    dm = np.concatenate([mag[:1], mag[1:] - mag[:-1]])
    thr = np.arange(NUM_BINS, dtype=np.float32) / NUM_BINS
    mrow = np.concatenate([ds, dm, thr]).astype(np.float32)
    meta = np.zeros((P, NMETA), dtype=np.float32)
    meta[:, : 3 * NUM_BINS] = mrow
    meta[np.arange(P), 3 * NUM_BINS + (np.arange(P) % G)] = 1.0

    in_maps = []
    for i in range(NCORES):
        shard = inp[NSHARD * i : NSHARD * (i + 1)]
        tsh = tgt[NSHARD * i : NSHARD * (i + 1)].reshape(R, P)  # [slot, partition]
        # chunk layout: within a k-slot chunk, partition-major so each
        # partition line is one contiguous 4*k*C-byte DMA descriptor
        xbuf = shard.copy()
        for s0, k in CHUNKS:
            if k > 1:
                xbuf[s0 * P : (s0 + k) * P] = (
                    shard[s0 * P : (s0 + k) * P]
                    .reshape(k, P, C)
                    .transpose(1, 0, 2)
                    .reshape(k * P, C)
                )
        # gather indices: slot s (at position j of its chunk) gathers
        # column j*C + target
        off = np.empty(R, dtype=np.int64)
        for s0, k in CHUNKS:
            for j in range(k):
                off[s0 + j] = j * C
        idxv = np.zeros((P, NIDX), dtype=np.uint16)
        for ci, (s0, k) in enumerate(CHUNKS):
            ip = IDX_POS[ci]
            idxv[:, ip : ip + k] = tsh[s0 : s0 + k].T + off[None, s0 : s0 + k]
        in_maps.append(
            {
                "x": np.ascontiguousarray(xbuf),
                "meta": meta,
                "idx": np.ascontiguousarray(idxv),
            }
        )
    return in_maps


def kernel(input, target, gammas, _trace=False, _tmpdir=None):
    from concourse.bass_utils import run_bass_kernel_spmd

    in_maps = _make_in_maps(input, target, gammas)
    res = run_bass_kernel_spmd(
        _get_nc(),
        in_maps,
        core_ids=list(range(NCORES)),
        trace=_trace,
        tmpdir=_tmpdir,
    )
    total = 0.0
    for i in range(NCORES):
        total += float(np.asarray(res.results[i]["out"], dtype=np.float64).sum())
    if _trace:
        kernel._last_result = res
    return np.array(-total, dtype=np.float32)
